# revision 1
# baseline (speedup 1.0000x reference)
"""Trainium2 Bass kernel for nn_Concat_26147760898611.

Mean-pool over the word dim of article_concat [256, 2048, 300] and
options_concat [256, 64, 300], concat features -> [256, 600].

Sharding: pure data parallel over batch across 8 NeuronCores
(32 batches per core). Per core:
  - each article batch [2048, 300] is DMA'd as one 2.46 MB transfer into
    an SBUF tile [128 partitions, 16 words, 300 feat] where partition p
    holds 16 *consecutive* words (fully contiguous 19.2 KB per
    partition -> line-rate DMA).
  - the word axis is folded FOLD_K times on the VectorEngine (fp32-exact
    adds); the surviving chunks are reduced across the partition dim on
    the TensorEngine with a ones-selector stationary operand whose
    single ones-column routes the sum into PSUM row b of a [32, 300]
    accumulator.
  - options: same tile shape; partition p holds 16 consecutive words of
    batch p//4, one block-selector matmul per surviving chunk reduces
    all 32 batches at once.
  - a burst of dummy matmuls at kernel start warms the PE HAM clock
    gate (1.2 -> 2.4 GHz) before real data lands.
  - the last batch is split into quarter tiles so the post-last-DMA
    tail (fold + matmul + scale + store) is short.
  - ScalarEngine applies the 1/n scaling into the [32, 600] output tile.

Self-contained: hardcodes all shapes; no file reads.
"""

import numpy as np

N_CORES = 8
B = 256  # full batch
BC = B // N_CORES  # 32 batches per core
DIM = 300
AW = 2048  # article words per batch
OW = 64  # options words per batch
P = 128  # SBUF partitions
AWP = AW // P  # 16 article words per partition
FOLD_K = 2  # DVE fold levels before the PE reduction
TAPER_FOLD_K = 3  # deeper fold for the last article batches (fewer cold
TAPER_START = 26  # PE passes in the tail; DVE has slack once DMA tapers)
TAIL_CHUNKS = [8, 4, 2, 1, 1]  # geometric split of the final batch
DATA_BUFS = 6
FOLD_BUFS = 3
WARMUP_MMS = 12
# float32r would stream the PE in one pass (vs fp32's two half-speed
# passes) but walrus requires the full producer chain to round to f32r
# and rejects this program; kept off.
USE_F32R = False

_CACHE = {}


def _build_nc():
    import concourse.bacc as bacc
    import concourse.mybir as mybir
    import concourse.tile as tile

    f32 = mybir.dt.float32
    f32mm = mybir.dt.float32r if USE_F32R else f32
    nc = bacc.Bacc("TRN2", target_bir_lowering=False, debug=False)

    art = nc.dram_tensor("article", [BC, AW, DIM], f32, kind="ExternalInput")
    opt = nc.dram_tensor("options", [BC, OW, DIM], f32, kind="ExternalInput")
    sel_a = nc.dram_tensor("sel_a", [P, 2 * BC - 1], f32mm, kind="ExternalInput")
    sel_o = nc.dram_tensor("sel_o", [P, BC], f32mm, kind="ExternalInput")
    out = nc.dram_tensor("out", [BC, 2 * DIM], f32, kind="ExternalOutput")

    # [BC, 128, 16, 300]: partition p <- words p*16 .. p*16+15 (contiguous)
    art_r = art.ap().rearrange("b (p w) f -> b p w f", p=P)
    # per-partition word view of the last batch: [128, 16, 300]
    art_last = art.ap()[BC - 1].rearrange("(p w) f -> p w f", p=P)
    # [128, 16, 300]: partition p <- 16 consecutive words of batch p//4
    opt_r = opt.ap().rearrange("b (s q) f -> (b s) q f", s=P // BC)

    with tile.TileContext(nc) as tc:
        with (
            tc.tile_pool(name="const", bufs=1) as cpool,
            tc.tile_pool(name="data", bufs=DATA_BUFS) as dpool,
            tc.tile_pool(name="fold", bufs=FOLD_BUFS) as fpool,
            tc.tile_pool(name="outp", bufs=1) as opool,
            tc.tile_pool(name="psum", bufs=1, space="PSUM") as ppool,
        ):
            sel_a_t = cpool.tile([P, 2 * BC - 1], f32mm, tag="sel_a")
            nc.sync.dma_start(sel_a_t[:], sel_a.ap()[:])
            sel_o_t = cpool.tile([P, BC], f32mm, tag="sel_o")
            nc.sync.dma_start(sel_o_t[:], sel_o.ap()[:])

            psum_a = ppool.tile([BC, DIM], f32, tag="psum_a")
            psum_b = ppool.tile([BC, DIM], f32, tag="psum_b")
            psum_w = ppool.tile([BC, 2 * BC - 1], f32, tag="psum_w")

            # PE warmup: flip the HAM clock gate to 2.4 GHz before the
            # first data tile lands. Results are never read.
            for _ in range(WARMUP_MMS):
                nc.tensor.matmul(
                    psum_w[:], sel_o_t[:], sel_a_t[:], start=True, stop=True
                )

            out_t = opool.tile([BC, 2 * DIM], f32, tag="out")

            def reduce_block(src_ap, nch, sel_ap, psum, first, last,
                             fold_k=FOLD_K):
                t = dpool.tile([P, nch, DIM], f32, tag="data")
                nc.sync.dma_start(t[:], src_ap)
                cur, n = t, nch
                for lvl in range(fold_k):
                    if n == 1:
                        break
                    n //= 2
                    nxt = fpool.tile([P, n, DIM], f32, tag=f"fold{lvl}_{nch}")
                    nc.vector.tensor_add(nxt[:], cur[:, 0:n, :], cur[:, n : 2 * n, :])
                    cur = nxt
                for j in range(n):
                    nc.tensor.matmul(
                        psum[:],
                        sel_ap,
                        cur[:, j, :],
                        start=(first and j == 0),
                        stop=(last and j == n - 1),
                    )

            # options first; drain its psum into the output tile early
            reduce_block(opt_r, AWP, sel_o_t[:], psum_b, True, True)
            nc.scalar.mul(out_t[:, DIM : 2 * DIM], psum_b[:], 1.0 / OW)

            for b in range(BC - 1):
                reduce_block(
                    art_r[b],
                    AWP,
                    sel_a_t[:, BC - 1 - b : 2 * BC - 1 - b],
                    psum_a,
                    b == 0,
                    False,
                    fold_k=TAPER_FOLD_K if b >= TAPER_START else FOLD_K,
                )
            # final batch in geometrically shrinking tiles -> the very
            # last DMA is tiny and its fold+matmul tail is short
            sel_last = sel_a_t[:, 0:BC]
            assert sum(TAIL_CHUNKS) == AWP
            w0 = 0
            for i, nch in enumerate(TAIL_CHUNKS):
                reduce_block(
                    art_last[:, w0 : w0 + nch, :],
                    nch,
                    sel_last,
                    psum_a,
                    False,
                    i == len(TAIL_CHUNKS) - 1,
                )
                w0 += nch

            nc.scalar.mul(out_t[:, 0:DIM], psum_a[:], 1.0 / AW)
            nc.sync.dma_start(out.ap()[:], out_t[:])

    nc.compile()
    return nc


def get_nc():
    if "nc" not in _CACHE:
        _CACHE["nc"] = _build_nc()
    return _CACHE["nc"]


def _sel_arrays():
    sel_a = np.zeros((P, 2 * BC - 1), np.float32)
    sel_a[:, BC - 1] = 1.0
    sel_o = np.zeros((P, BC), np.float32)
    sel_o[np.arange(P), np.arange(P) // (P // BC)] = 1.0
    return sel_a, sel_o


def make_in_maps(article, options):
    article = np.ascontiguousarray(np.asarray(article, dtype=np.float32))
    options = np.ascontiguousarray(np.asarray(options, dtype=np.float32))
    assert article.shape == (B, AW, DIM), article.shape
    assert options.shape == (B, OW, DIM), options.shape
    sel_a, sel_o = _sel_arrays()
    return [
        {
            "article": article[i * BC : (i + 1) * BC],
            "options": options[i * BC : (i + 1) * BC],
            "sel_a": sel_a,
            "sel_o": sel_o,
        }
        for i in range(N_CORES)
    ]


def run_sharded(article, options, **spmd_kwargs):
    from concourse.bass_utils import run_bass_kernel_spmd

    nc = get_nc()
    in_maps = make_in_maps(article, options)
    res = run_bass_kernel_spmd(nc, in_maps, list(range(N_CORES)), **spmd_kwargs)
    full = np.concatenate(
        [res.results[i]["out"] for i in range(N_CORES)], axis=0
    ).astype(np.float32)
    return full, res


def kernel(article_concat, options_concat):
    full, _ = run_sharded(article_concat, options_concat)
    return full

